# revision 1
# baseline (speedup 1.0000x reference)
"""Multi-head cross-attention on 8 Trainium2 NeuronCores.

Problem shapes (hardcoded): B=4, Ld=1024, Le=2048, d_model=1024, 8 heads x 128.
Sharding: core c handles batch b=c//2 and head-group g=c%2 (4 heads each).
Each core computes q/k/v projections for its heads, attention, and a partial
output projection over its heads' value dims; the host sums the two partial
outputs per batch and adds b_o.

All matmuls run as float32r (fp32 storage, full-rate PE streaming).
Softmax denominators come from a ones-column matmul accumulated in PSUM
alongside the attention*V matmul; normalization multiplies by the
partition-broadcast reciprocal.

Phase order is chosen so DMA stays ahead of the PE: the K projection (which
owns the cold start) streams its weight+encoder chunks per contraction step,
later phases' inputs trickle in behind the compute of earlier ones, and the
Q projection runs last on fully resident inputs. PSUM is ping-ponged in
4-bank groups so accumulator copy-backs overlap the next group's matmuls.
The output projection is interleaved per query-half behind the attention
loop.
"""

import math
import sys

import numpy as np

for _p in ("/opt/trn_rl_repo", "/root/.axon_site/_ro/trn_rl_repo"):
    if _p not in sys.path:
        sys.path.append(_p)

B = 4
LQ = 1024
LK = 2048
D = 1024
H = 8
DH = 128
P = 128
HPC = 4          # heads per core
OQ = HPC * DH    # 512 projected dims per core
NQ = 512         # matmul moving free dim
KC = D // P      # 8 contraction chunks for projections
LKC = LK // P    # 16 key chunks
N_CORES = 8

_BUILT = {}


def _build(masked):
    import concourse.bass as bass  # noqa: F401
    import concourse.tile as tile
    import concourse.mybir as mybir
    from concourse import bacc

    f32 = mybir.dt.float32
    f32r = mybir.dt.float32r
    Exp = mybir.ActivationFunctionType.Exp

    nc = bacc.Bacc("TRN2", target_bir_lowering=False, debug=False,
                   num_devices=N_CORES)

    xT = nc.dram_tensor("xT", [D, LQ], f32r, kind="ExternalInput").ap()
    encT = nc.dram_tensor("encT", [D, LK], f32r, kind="ExternalInput").ap()
    wqT = nc.dram_tensor("wqT", [D, OQ], f32r, kind="ExternalInput").ap()
    wkT = nc.dram_tensor("wkT", [D, OQ], f32r, kind="ExternalInput").ap()
    wvT = nc.dram_tensor("wvT", [D, OQ], f32r, kind="ExternalInput").ap()
    woT = nc.dram_tensor("woT", [OQ, D], f32r, kind="ExternalInput").ap()
    bq_d = nc.dram_tensor("bq", [P, HPC], f32, kind="ExternalInput").ap()
    bk_d = nc.dram_tensor("bk", [P, HPC], f32, kind="ExternalInput").ap()
    bv_d = nc.dram_tensor("bv", [P, HPC], f32, kind="ExternalInput").ap()
    ones_d = nc.dram_tensor("ones", [P, 1], f32r, kind="ExternalInput").ap()
    if masked:
        maskT = nc.dram_tensor("maskT", [LK, LQ], f32, kind="ExternalInput").ap()
    out_d = nc.dram_tensor("out", [LQ, D], f32, kind="ExternalOutput").ap()

    HLK = LK // 2  # 1024, one lk-half of the encoder

    with tile.TileContext(nc) as tc:
        with tc.tile_pool(name="persist", bufs=1) as persist:
            qT = [persist.tile([P, LQ], f32r, name=f"qT{h}") for h in range(HPC)]
            kT = [persist.tile([P, LK], f32r, name=f"kT{h}") for h in range(HPC)]
            vch = [persist.tile([P, OQ], f32r, name=f"v{j}") for j in range(LKC)]
            bq_sb = persist.tile([P, HPC], f32, name="bq")
            bk_sb = persist.tile([P, HPC], f32, name="bk")
            bv_sb = persist.tile([P, HPC], f32, name="bv")
            ones_col = persist.tile([P, 1], f32r, name="ones")
            wkc = [persist.tile([P, OQ], f32r, name=f"wk{d}") for d in range(KC)]
            wvc = [persist.tile([P, OQ], f32r, name=f"wv{d}") for d in range(KC)]
            wqc = [persist.tile([P, OQ], f32r, name=f"wq{d}") for d in range(KC)]
            woch = [persist.tile([P, D], f32r, name=f"wo{h}")
                    for h in range(HPC)]

            with (
                tc.tile_pool(name="acc", bufs=1, space="PSUM") as acc,
                tc.tile_pool(name="xh", bufs=6) as xhp,
            ):
                banks = [acc.tile([P, NQ], f32, name=f"bank{t}")
                         for t in range(8)]

                def kproj(e, lh, grp):
                    """kT for one lk-half: grp 0 -> banks 0-3, grp 1 -> 4-7."""
                    l2 = grp
                    for d in range(KC):
                        for h in range(HPC):
                            nc.tensor.matmul(
                                banks[grp * 4 + h][:],
                                wkc[d][:, h * DH:(h + 1) * DH],
                                e[d][:, l2 * NQ:(l2 + 1) * NQ],
                                start=(d == 0), stop=(d == KC - 1))
                    for h in range(HPC):
                        off = lh * HLK + l2 * NQ
                        nc.vector.tensor_scalar_add(
                            kT[h][:, off:off + NQ], banks[grp * 4 + h][:],
                            bk_sb[:, h:h + 1])

                def vproj(e, lh, grp):
                    """v chunks j = lh*8 + grp*4 ... +4."""
                    for d in range(KC):
                        for jj in range(4):
                            jloc = grp * 4 + jj
                            nc.tensor.matmul(
                                banks[grp * 4 + jj][:],
                                e[d][:, jloc * P:(jloc + 1) * P],
                                wvc[d][:],
                                start=(d == 0), stop=(d == KC - 1))
                    for jj in range(4):
                        nc.vector.tensor_copy(
                            vch[lh * 8 + grp * 4 + jj][:],
                            banks[grp * 4 + jj][:])

                def qproj(grp):
                    """qT for query half q2=grp from the streamed x half."""
                    q2 = grp
                    xh = []
                    for d in range(KC):
                        xt = xhp.tile([P, NQ], f32r, name="xh")
                        nc.sync.dma_start(
                            xt[:], xT[d * P:(d + 1) * P,
                                      q2 * NQ:(q2 + 1) * NQ])
                        xh.append(xt)
                        for h in range(HPC):
                            nc.tensor.matmul(
                                banks[grp * 4 + h][:],
                                wqc[d][:, h * DH:(h + 1) * DH],
                                xt[:],
                                start=(d == 0), stop=(d == KC - 1))
                    for h in range(HPC):
                        nc.scalar.add(
                            qT[h][:, q2 * NQ:(q2 + 1) * NQ],
                            banks[grp * 4 + h][:], bq_sb[:, h:h + 1])

                with tc.tile_pool(name="enc0", bufs=1) as enc0p:
                    e0 = [enc0p.tile([P, HLK], f32r, name=f"e0_{d}")
                          for d in range(KC)]
                    # --- K proj, lk-half 0 (cold start: stream wk + e0).
                    for d in range(KC):
                        nc.sync.dma_start(wkc[d][:],
                                          wkT[d * P:(d + 1) * P, :])
                        nc.sync.dma_start(e0[d][:],
                                          encT[d * P:(d + 1) * P, :HLK])
                        if d == 0:
                            nc.sync.dma_start(bq_sb[:], bq_d[:])
                            nc.sync.dma_start(bk_sb[:], bk_d[:])
                            nc.sync.dma_start(bv_sb[:], bv_d[:])
                            nc.sync.dma_start(ones_col[:], ones_d[:])
                    kproj(e0, 0, 0)
                    kproj(e0, 0, 1)
                    # --- V proj, lk-half 0; wv streams in behind.
                    for d in range(KC):
                        nc.sync.dma_start(wvc[d][:],
                                          wvT[d * P:(d + 1) * P, :])
                    vproj(e0, 0, 0)
                    vproj(e0, 0, 1)

                with tc.tile_pool(name="enc1", bufs=1) as enc1p:
                    e1 = [enc1p.tile([P, HLK], f32r, name=f"e1_{d}")
                          for d in range(KC)]
                    # --- K proj, lk-half 1 (e1 streams per d).
                    for d in range(KC):
                        nc.sync.dma_start(e1[d][:],
                                          encT[d * P:(d + 1) * P, HLK:])
                    kproj(e1, 1, 0)
                    kproj(e1, 1, 1)
                    # --- V proj, lk-half 1; wq + wo stream in behind.
                    for d in range(KC):
                        nc.sync.dma_start(wqc[d][:],
                                          wqT[d * P:(d + 1) * P, :])
                        if d % 2 == 0:
                            nc.sync.dma_start(woch[d // 2][:],
                                              woT[(d // 2) * P:
                                                  (d // 2 + 1) * P, :])
                    vproj(e1, 1, 0)
                    vproj(e1, 1, 1)
                    # --- Q proj (x halves stream inside).
                    qproj(0)
                    qproj(1)

            # ---- Attention (q2-outer) + interleaved output projection.
            with tc.tile_pool(name="att", bufs=1) as attp:
                valsT = [attp.tile([P, LQ], f32r, name=f"valsT{h}")
                         for h in range(HPC)]

                with (
                    tc.tile_pool(name="pTp", bufs=8) as pTp,
                    tc.tile_pool(name="smallp", bufs=2) as smallp,
                    tc.tile_pool(name="maskp", bufs=16 if masked else 1) as maskp,
                    tc.tile_pool(name="osb", bufs=4) as osb,
                    tc.tile_pool(name="pss", bufs=3, space="PSUM") as pss,
                    tc.tile_pool(name="psa", bufs=2, space="PSUM") as psa,
                    tc.tile_pool(name="psd", bufs=1, space="PSUM") as psd,
                    tc.tile_pool(name="pso", bufs=2, space="PSUM") as pso,
                ):
                    for q2 in range(LQ // NQ):
                        if masked:
                            mch = []
                            for j in range(LKC):
                                mt = maskp.tile([P, NQ], f32, name=f"m{j}")
                                nc.sync.dma_start(
                                    mt[:],
                                    maskT[j * P:(j + 1) * P,
                                          q2 * NQ:(q2 + 1) * NQ])
                                mch.append(mt)
                        for h in range(HPC):
                            ps_v = psa.tile([P, NQ], f32, name="ps_v")
                            ps_d = psd.tile([1, NQ], f32, name="ps_d")
                            for j in range(LKC):
                                ps_s = pss.tile([P, NQ], f32, name="ps_s")
                                nc.tensor.matmul(
                                    ps_s[:],
                                    kT[h][:, j * P:(j + 1) * P],
                                    qT[h][:, q2 * NQ:(q2 + 1) * NQ],
                                    start=True, stop=True)
                                pT = pTp.tile([P, NQ], f32r, name="pT")
                                if masked:
                                    nc.vector.tensor_add(
                                        ps_s[:], ps_s[:], mch[j][:])
                                nc.scalar.activation(pT[:], ps_s[:], Exp)
                                nc.tensor.matmul(
                                    ps_v[:],
                                    vch[j][:, h * DH:(h + 1) * DH],
                                    pT[:],
                                    start=(j == 0), stop=(j == LKC - 1))
                                nc.tensor.matmul(
                                    ps_d[:],
                                    ones_col[:],
                                    pT[:],
                                    start=(j == 0), stop=(j == LKC - 1))
                            recip = smallp.tile([1, NQ], f32, name="recip")
                            nc.vector.reciprocal(recip[:], ps_d[:])
                            bcast = smallp.tile([P, NQ], f32, name="bcast")
                            nc.gpsimd.partition_broadcast(bcast[:], recip[:])
                            vs = valsT[h][:, q2 * NQ:(q2 + 1) * NQ]
                            nc.vector.tensor_mul(vs, ps_v[:], bcast[:])
                            nc.scalar.add(vs, vs, bv_sb[:, h:h + 1])
                        # Output projection for this query half.
                        for lqc in range(q2 * 4, (q2 + 1) * 4):
                            for o2 in range(D // NQ):
                                po = pso.tile([P, NQ], f32, name="pso")
                                for h in range(HPC):
                                    nc.tensor.matmul(
                                        po[:],
                                        valsT[h][:, lqc * P:(lqc + 1) * P],
                                        woch[h][:, o2 * NQ:(o2 + 1) * NQ],
                                        start=(h == 0), stop=(h == HPC - 1))
                                ot = osb.tile([P, NQ], f32, name="ot")
                                nc.vector.tensor_copy(ot[:], po[:])
                                nc.sync.dma_start(
                                    out_d[lqc * P:(lqc + 1) * P,
                                          o2 * NQ:(o2 + 1) * NQ], ot[:])

    nc.compile()
    return nc


def _get_built(masked):
    if masked not in _BUILT:
        _BUILT[masked] = _build(masked)
    return _BUILT[masked]


def _shard_inputs(inputs, masked):
    x = np.asarray(inputs["mhca_input"], np.float32)
    enc = np.asarray(inputs["encoder_output"], np.float32)
    mask = np.asarray(inputs["cross_mask"], np.float32)
    W_kv = np.asarray(inputs["W_kv"], np.float32)
    b_kv = np.asarray(inputs["b_kv"], np.float32)
    W_q = np.asarray(inputs["W_q"], np.float32)
    b_q = np.asarray(inputs["b_q"], np.float32)
    W_o = np.asarray(inputs["W_o"], np.float32)

    scale = 1.0 / math.sqrt(DH)
    in_maps = []
    for c in range(N_CORES):
        b = c // 2
        g = c % 2
        heads = list(range(g * HPC, (g + 1) * HPC))
        sl = slice(g * OQ, (g + 1) * OQ)
        k_rows = np.concatenate(
            [W_kv[h * 2 * DH:h * 2 * DH + DH] for h in heads], 0)
        v_rows = np.concatenate(
            [W_kv[h * 2 * DH + DH:(h + 1) * 2 * DH] for h in heads], 0)
        m = {
            "xT": np.ascontiguousarray(x[b].T),
            "encT": np.ascontiguousarray(enc[b].T),
            "wqT": np.ascontiguousarray((W_q[sl] * scale).T),
            "wkT": np.ascontiguousarray(k_rows.T),
            "wvT": np.ascontiguousarray(v_rows.T),
            "woT": np.ascontiguousarray(W_o[:, sl].T),
            "bq": np.ascontiguousarray((b_q[sl] * scale).reshape(HPC, DH).T),
            "bk": np.ascontiguousarray(
                np.stack([b_kv[h * 2 * DH:h * 2 * DH + DH] for h in heads], 1)),
            "bv": np.ascontiguousarray(
                np.stack([b_kv[h * 2 * DH + DH:(h + 1) * 2 * DH]
                          for h in heads], 1)),
            "ones": np.ones((P, 1), np.float32),
        }
        if masked:
            m["maskT"] = np.ascontiguousarray(mask[b].T)
        in_maps.append(m)
    return in_maps


def kernel(mhca_input, encoder_output, cross_mask, W_kv, b_kv, W_q, b_q, W_o,
           b_o):
    from concourse.bass_utils import run_bass_kernel_spmd

    inputs = {
        "mhca_input": mhca_input, "encoder_output": encoder_output,
        "cross_mask": cross_mask, "W_kv": W_kv, "b_kv": b_kv, "W_q": W_q,
        "b_q": b_q, "W_o": W_o,
    }
    b_o = np.asarray(b_o, np.float32)
    masked = bool(np.any(np.asarray(cross_mask)))
    nc = _get_built(masked)
    in_maps = _shard_inputs(inputs, masked)

    res = run_bass_kernel_spmd(nc, in_maps, core_ids=list(range(N_CORES)))
    outs = [res.results[c]["out"] for c in range(N_CORES)]
    full = np.stack([outs[2 * b] + outs[2 * b + 1] for b in range(B)], 0)
    return (full + b_o[None, None, :]).astype(np.float32)



# revision 2
# speedup vs baseline: 1.1812x; 1.1812x over previous
"""Multi-head cross-attention on 8 Trainium2 NeuronCores.

Problem shapes (hardcoded): B=4, Ld=1024, Le=2048, d_model=1024, 8 heads x 128.
Sharding: core c handles batch b=c//2 and head-group g=c%2 (4 heads each).
Each core computes q/k/v projections for its heads, attention, and a partial
output projection over its heads' value dims; the host sums the two partial
outputs per batch and adds the (bias-folded) output bias.

Key scheduling/engine decisions:
- Projection inputs (enc, x, wq, wk, wv) ship as bf16: halves DMA bytes so
  the cold start is PE-bound, at identical PE throughput.
- Projections run d-major across all 8 PSUM banks; bank copy-backs alternate
  DVE/Act so the next phase's bank-0 chain never waits.
- Softmax denominators never touch the PE: exp'd score chunks (bf16)
  accumulate on DVE (2x mode), then one gpsimd partition_all_reduce per
  (q-half, head) produces the broadcast denominator, reciprocal on DVE.
- exp processes two key-chunks per Act instruction ([128,1024] spanning two
  PSUM banks) to keep Act throughput above the PE's score+AV rate.
- b_v is folded into the host-side output bias (b_eff = b_o + W_o @ b_v),
  removing all bias work from the attention loop.
- Attention is software-pipelined: one filler matmul per inner step keeps
  the PE busy while Act computes exp. Fillers for q-half 0 are the second
  half of the Q projection; fillers for q-half 1 are q-half 0's output
  projection.
"""

import math
import sys

import numpy as np

for _p in ("/opt/trn_rl_repo", "/root/.axon_site/_ro/trn_rl_repo"):
    if _p not in sys.path:
        sys.path.append(_p)

B = 4
LQ = 1024
LK = 2048
D = 1024
H = 8
DH = 128
P = 128
HPC = 4          # heads per core
OQ = HPC * DH    # 512 projected dims per core
NQ = 512         # matmul moving free dim
KC = D // P      # 8 contraction chunks for projections
LKC = LK // P    # 16 key chunks
HLK = LK // 2    # 1024, one lk-half of the encoder
N_CORES = 8

_BUILT = {}


def _build(masked):
    import concourse.bass as bass  # noqa: F401
    import concourse.tile as tile
    import concourse.mybir as mybir
    from concourse import bacc
    from concourse import bass_isa

    f32 = mybir.dt.float32
    f32r = mybir.dt.float32r
    bf16 = mybir.dt.bfloat16
    Exp = mybir.ActivationFunctionType.Exp

    nc = bacc.Bacc("TRN2", target_bir_lowering=False, debug=False,
                   num_devices=N_CORES)

    xT = nc.dram_tensor("xT", [D, LQ], bf16, kind="ExternalInput").ap()
    encT = nc.dram_tensor("encT", [D, LK], bf16, kind="ExternalInput").ap()
    wqT = nc.dram_tensor("wqT", [D, OQ], bf16, kind="ExternalInput").ap()
    wkT = nc.dram_tensor("wkT", [D, OQ], bf16, kind="ExternalInput").ap()
    wvT = nc.dram_tensor("wvT", [D, OQ], bf16, kind="ExternalInput").ap()
    woT = nc.dram_tensor("woT", [OQ, D], f32r, kind="ExternalInput").ap()
    bq_d = nc.dram_tensor("bq", [P, HPC], f32, kind="ExternalInput").ap()
    bk_d = nc.dram_tensor("bk", [P, HPC], f32, kind="ExternalInput").ap()
    if masked:
        maskT = nc.dram_tensor("maskT", [LK, LQ], f32, kind="ExternalInput").ap()
    out_d = nc.dram_tensor("out", [LQ, D], f32, kind="ExternalOutput").ap()

    with tile.TileContext(nc) as tc:
        with tc.tile_pool(name="persist", bufs=1) as persist:
            kT = [persist.tile([P, LK], f32r, name=f"kT{h}") for h in range(HPC)]
            qT = [persist.tile([P, LQ], f32r, name=f"qT{h}") for h in range(HPC)]
            vch = [persist.tile([P, OQ], bf16, name=f"v{j}") for j in range(LKC)]
            wqc = [persist.tile([P, OQ], bf16, name=f"wq{d}") for d in range(KC)]
            woch = [persist.tile([P, D], f32r, name=f"wo{h}") for h in range(HPC)]
            bq_sb = persist.tile([P, HPC], f32, name="bq")
            bk_sb = persist.tile([P, HPC], f32, name="bk")
            warm = persist.tile([P, 256], bf16, name="warm")

            with (
                tc.tile_pool(name="acc", bufs=1, space="PSUM") as accp,
                tc.tile_pool(name="wk", bufs=1) as wkp,
                tc.tile_pool(name="wv", bufs=1) as wvp,
                tc.tile_pool(name="xh", bufs=8) as xhp,
            ):
                banks = [accp.tile([P, NQ], f32, name=f"bank{t}")
                         for t in range(8)]
                wkc = [wkp.tile([P, OQ], bf16, name=f"wk{d}") for d in range(KC)]
                wvc = [wvp.tile([P, OQ], bf16, name=f"wv{d}") for d in range(KC)]

                # --- PE warm-up: memset a tile (no DMA) and run dummy
                # matmuls so the p-state ramp happens before real work.
                nc.vector.memset(warm[:], 1.0)
                for _ in range(14):
                    nc.tensor.matmul(banks[0][0:16, 0:256], warm[:, 0:16],
                                     warm[:], start=True, stop=True)

                def kproj_half(e, lh):
                    for d in range(KC):
                        for g in range(2):
                            for h in range(HPC):
                                nc.tensor.matmul(
                                    banks[g * 4 + h][:],
                                    wkc[d][:, h * DH:(h + 1) * DH],
                                    e[d][:, g * NQ:(g + 1) * NQ],
                                    start=(d == 0), stop=(d == KC - 1))
                    for i in range(8):
                        g, h = i // 4, i % 4
                        dst = kT[h][:, lh * HLK + g * NQ:lh * HLK + (g + 1) * NQ]
                        if i % 2 == 0:
                            nc.vector.tensor_scalar_add(
                                dst, banks[i][:], bk_sb[:, h:h + 1])
                        else:
                            nc.scalar.add(dst, banks[i][:], bk_sb[:, h:h + 1])

                def vproj_half(e, lh):
                    for d in range(KC):
                        for j8 in range(8):
                            nc.tensor.matmul(
                                banks[j8][:],
                                e[d][:, j8 * P:(j8 + 1) * P],
                                wvc[d][:],
                                start=(d == 0), stop=(d == KC - 1))
                    for j8 in range(8):
                        dst = vch[lh * 8 + j8][:]
                        if j8 % 2 == 0:
                            nc.vector.tensor_copy(dst, banks[j8][:])
                        else:
                            nc.scalar.copy(dst, banks[j8][:])

                with tc.tile_pool(name="enc0", bufs=1) as enc0p:
                    e0 = [enc0p.tile([P, HLK], bf16, name=f"e0_{d}")
                          for d in range(KC)]
                    # K proj, lk-half 0 (cold start: stream wk + e0 per d).
                    for d in range(KC):
                        nc.sync.dma_start(wkc[d][:],
                                          wkT[d * P:(d + 1) * P, :])
                        nc.sync.dma_start(e0[d][:],
                                          encT[d * P:(d + 1) * P, :HLK])
                        if d == 0:
                            nc.sync.dma_start(bq_sb[:], bq_d[:])
                            nc.sync.dma_start(bk_sb[:], bk_d[:])
                    kproj_half(e0, 0)
                    # V proj, lk-half 0; wv streams in behind.
                    for d in range(KC):
                        nc.sync.dma_start(wvc[d][:],
                                          wvT[d * P:(d + 1) * P, :])
                    vproj_half(e0, 0)

                with tc.tile_pool(name="enc1", bufs=1) as enc1p:
                    e1 = [enc1p.tile([P, HLK], bf16, name=f"e1_{d}")
                          for d in range(KC)]
                    for d in range(KC):
                        nc.sync.dma_start(e1[d][:],
                                          encT[d * P:(d + 1) * P, HLK:])
                    kproj_half(e1, 1)
                    for d in range(KC):
                        nc.sync.dma_start(wqc[d][:],
                                          wqT[d * P:(d + 1) * P, :])
                    vproj_half(e1, 1)

                    # Q proj, query half 0, in two 2-head subphases so the
                    # head-0/1 copy-backs finish before attention needs qT.
                    xg0 = []
                    for d in range(KC):
                        xt = xhp.tile([P, NQ], bf16, name="xh")
                        nc.sync.dma_start(xt[:], xT[d * P:(d + 1) * P, 0:NQ])
                        xg0.append(xt)
                    for sub in range(2):
                        hs = (0, 1) if sub == 0 else (2, 3)
                        for d in range(KC):
                            for h in hs:
                                nc.tensor.matmul(
                                    banks[h][:],
                                    wqc[d][:, h * DH:(h + 1) * DH],
                                    xg0[d][:],
                                    start=(d == 0), stop=(d == KC - 1))
                        for h in hs:
                            nc.scalar.add(qT[h][:, 0:NQ], banks[h][:],
                                          bq_sb[:, h:h + 1])
                    # wo streams in for the output projection.
                    for h in range(HPC):
                        nc.sync.dma_start(woch[h][:],
                                          woT[h * P:(h + 1) * P, :])

            # ---- Attention, software-pipelined with filler matmuls.
            with tc.tile_pool(name="att", bufs=1) as attp:
                valsT = [attp.tile([P, LQ], f32r, name=f"valsT{h}")
                         for h in range(HPC)]
                xg1 = [attp.tile([P, NQ], bf16, name=f"xg1_{d}")
                       for d in range(KC)]

                with (
                    tc.tile_pool(name="pTp", bufs=6) as pTp,
                    tc.tile_pool(name="dnp", bufs=2) as dnp,
                    tc.tile_pool(name="maskp", bufs=16 if masked else 1) as maskp,
                    tc.tile_pool(name="osb", bufs=4) as osb,
                    tc.tile_pool(name="pss", bufs=2, space="PSUM") as pss,
                    tc.tile_pool(name="psa", bufs=2, space="PSUM") as psa,
                ):
                    for d in range(KC):
                        nc.sync.dma_start(xg1[d][:],
                                          xT[d * P:(d + 1) * P, NQ:2 * NQ])

                    def attn_q2(q2, fillers, out_pool):
                        """h-loops for one query half. fillers: list of
                        callables, each emitting one PE matmul (+ its own
                        non-PE follow-ups); consumed one per inner step."""
                        fi = [0]

                        def fill():
                            if fi[0] < len(fillers):
                                fillers[fi[0]]()
                                fi[0] += 1

                        if masked:
                            mch = []
                            for j in range(LKC):
                                mt = maskp.tile([P, NQ], f32, name=f"m{j}")
                                nc.sync.dma_start(
                                    mt[:], maskT[j * P:(j + 1) * P,
                                                 q2 * NQ:(q2 + 1) * NQ])
                                mch.append(mt)

                        for h in range(HPC):
                            qs = qT[h][:, q2 * NQ:(q2 + 1) * NQ]
                            ps_v = psa.tile([P, NQ], f32, name="ps_v")
                            pT = [None] * 8
                            acc = None

                            def spair(g):
                                t = pss.tile([P, 2 * NQ], f32, name="ps_s")
                                for jj in range(2):
                                    j = 2 * g + jj
                                    nc.tensor.matmul(
                                        t[:, jj * NQ:(jj + 1) * NQ],
                                        kT[h][:, j * P:(j + 1) * P],
                                        qs, start=True, stop=True)
                                return t

                            def do_exp(g, t):
                                if masked:
                                    for jj in range(2):
                                        j = 2 * g + jj
                                        nc.vector.tensor_add(
                                            t[:, jj * NQ:(jj + 1) * NQ],
                                            t[:, jj * NQ:(jj + 1) * NQ],
                                            mch[j][:])
                                p = pTp.tile([P, 2 * NQ], bf16, name="pT")
                                nc.scalar.activation(p[:], t[:], Exp)
                                pT[g] = p

                            def avpair(g):
                                for jj in range(2):
                                    j = 2 * g + jj
                                    nc.tensor.matmul(
                                        ps_v[:],
                                        vch[j][:, h * DH:(h + 1) * DH],
                                        pT[g][:, jj * NQ:(jj + 1) * NQ],
                                        start=(j == 0), stop=(j == LKC - 1))

                            st = [spair(0)]
                            st.append(spair(1))
                            for g in range(8):
                                do_exp(g, st[g])
                                if g + 2 < 8:
                                    st.append(spair(g + 2))
                                fill()
                                avpair(g)
                                # denominator accumulation on DVE (bf16 2x)
                                p = pT[g]
                                if g == 0:
                                    acc = dnp.tile([P, NQ], bf16, name="dacc")
                                    nc.vector.tensor_add(
                                        acc[:], p[:, 0:NQ], p[:, NQ:2 * NQ])
                                else:
                                    nc.vector.tensor_add(
                                        acc[:], acc[:], p[:, 0:NQ])
                                    nc.vector.tensor_add(
                                        acc[:], acc[:], p[:, NQ:2 * NQ])
                            dbc = dnp.tile([P, NQ], f32, name="dbc")
                            nc.gpsimd.partition_all_reduce(
                                dbc[:], acc[:], channels=P,
                                reduce_op=bass_isa.ReduceOp.add)
                            rr = dnp.tile([P, NQ], f32, name="rr")
                            nc.vector.reciprocal(rr[:], dbc[:])
                            nc.vector.tensor_mul(
                                valsT[h][:, q2 * NQ:(q2 + 1) * NQ],
                                ps_v[:], rr[:])

                    def outproj_fillers(q2, out_pool):
                        """32 closures: 8 chains x 4 head-parts; the last
                        part of each chain emits the copy-back + store."""
                        fillers = []
                        state = {}
                        for c in range(8):
                            lqc = q2 * 4 + c // 2
                            o2 = c % 2
                            for hh in range(HPC):
                                def f(c=c, lqc=lqc, o2=o2, hh=hh):
                                    if hh == 0:
                                        state[c] = out_pool.tile(
                                            [P, NQ], f32, name="pso")
                                    po = state[c]
                                    nc.tensor.matmul(
                                        po[:],
                                        valsT[hh][:, lqc * P:(lqc + 1) * P],
                                        woch[hh][:, o2 * NQ:(o2 + 1) * NQ],
                                        start=(hh == 0), stop=(hh == HPC - 1))
                                    if hh == HPC - 1:
                                        ot = osb.tile([P, NQ], f32, name="ot")
                                        nc.vector.tensor_copy(ot[:], po[:])
                                        nc.sync.dma_start(
                                            out_d[lqc * P:(lqc + 1) * P,
                                                  o2 * NQ:(o2 + 1) * NQ],
                                            ot[:])
                                fillers.append(f)
                        return fillers

                    def qg1_fillers(psq):
                        """32 closures: 4 head d-chains; last d emits the
                        qT copy-back (DVE, since Act is saturated)."""
                        fillers = []
                        state = {}
                        for hh in range(HPC):
                            for d in range(KC):
                                def f(hh=hh, d=d):
                                    if d == 0:
                                        state[hh] = psq.tile(
                                            [P, NQ], f32, name="psq")
                                    pq = state[hh]
                                    nc.tensor.matmul(
                                        pq[:],
                                        wqc[d][:, hh * DH:(hh + 1) * DH],
                                        xg1[d][:],
                                        start=(d == 0), stop=(d == KC - 1))
                                    if d == KC - 1:
                                        nc.vector.tensor_scalar_add(
                                            qT[hh][:, NQ:2 * NQ], pq[:],
                                            bq_sb[:, hh:hh + 1])
                                fillers.append(f)
                        return fillers

                    with tc.tile_pool(name="psq", bufs=2, space="PSUM") as psq:
                        attn_q2(0, qg1_fillers(psq), None)
                    with tc.tile_pool(name="pso", bufs=2, space="PSUM") as pso:
                        attn_q2(1, outproj_fillers(0, pso), pso)
                        # output projection for query half 1, inline.
                        for f in outproj_fillers(1, pso):
                            f()

    nc.compile()
    return nc


def _get_built(masked):
    if masked not in _BUILT:
        _BUILT[masked] = _build(masked)
    return _BUILT[masked]


def _shard_inputs(inputs, masked):
    import ml_dtypes

    bf16 = ml_dtypes.bfloat16

    x = np.asarray(inputs["mhca_input"], np.float32)
    enc = np.asarray(inputs["encoder_output"], np.float32)
    mask = np.asarray(inputs["cross_mask"], np.float32)
    W_kv = np.asarray(inputs["W_kv"], np.float32)
    b_kv = np.asarray(inputs["b_kv"], np.float32)
    W_q = np.asarray(inputs["W_q"], np.float32)
    b_q = np.asarray(inputs["b_q"], np.float32)
    W_o = np.asarray(inputs["W_o"], np.float32)

    scale = 1.0 / math.sqrt(DH)
    in_maps = []
    for c in range(N_CORES):
        b = c // 2
        g = c % 2
        heads = list(range(g * HPC, (g + 1) * HPC))
        sl = slice(g * OQ, (g + 1) * OQ)
        k_rows = np.concatenate(
            [W_kv[h * 2 * DH:h * 2 * DH + DH] for h in heads], 0)
        v_rows = np.concatenate(
            [W_kv[h * 2 * DH + DH:(h + 1) * 2 * DH] for h in heads], 0)
        m = {
            "xT": np.ascontiguousarray(x[b].T).astype(bf16),
            "encT": np.ascontiguousarray(enc[b].T).astype(bf16),
            "wqT": np.ascontiguousarray((W_q[sl] * scale).T).astype(bf16),
            "wkT": np.ascontiguousarray(k_rows.T).astype(bf16),
            "wvT": np.ascontiguousarray(v_rows.T).astype(bf16),
            "woT": np.ascontiguousarray(W_o[:, sl].T),
            "bq": np.ascontiguousarray((b_q[sl] * scale).reshape(HPC, DH).T),
            "bk": np.ascontiguousarray(
                np.stack([b_kv[h * 2 * DH:h * 2 * DH + DH] for h in heads], 1)),
        }
        if masked:
            m["maskT"] = np.ascontiguousarray(mask[b].T)
        in_maps.append(m)
    return in_maps


def kernel(mhca_input, encoder_output, cross_mask, W_kv, b_kv, W_q, b_q, W_o,
           b_o):
    from concourse.bass_utils import run_bass_kernel_spmd

    inputs = {
        "mhca_input": mhca_input, "encoder_output": encoder_output,
        "cross_mask": cross_mask, "W_kv": W_kv, "b_kv": b_kv, "W_q": W_q,
        "b_q": b_q, "W_o": W_o,
    }
    b_kv = np.asarray(b_kv, np.float32)
    b_o = np.asarray(b_o, np.float32)
    W_o_np = np.asarray(W_o, np.float32)
    # v-bias folds into the output bias: out += W_o @ b_v + b_o
    b_v_vec = np.concatenate(
        [b_kv[h * 2 * DH + DH:(h + 1) * 2 * DH] for h in range(H)], 0)
    b_eff = b_o + W_o_np @ b_v_vec
    masked = bool(np.any(np.asarray(cross_mask)))
    nc = _get_built(masked)
    in_maps = _shard_inputs(inputs, masked)

    res = run_bass_kernel_spmd(nc, in_maps, core_ids=list(range(N_CORES)))
    outs = [res.results[c]["out"] for c in range(N_CORES)]
    full = np.stack([outs[2 * b] + outs[2 * b + 1] for b in range(B)], 0)
    return (full + b_eff[None, None, :]).astype(np.float32)


# revision 3
# speedup vs baseline: 1.2845x; 1.0875x over previous
"""Multi-head cross-attention on 8 Trainium2 NeuronCores.

Problem shapes (hardcoded): B=4, Ld=1024, Le=2048, d_model=1024, 8 heads x 128.
Sharding: core c handles batch b=c//2 and head-group g=c%2 (4 heads each).
Each core computes q/k/v projections for its heads, attention, and a partial
output projection over its heads' value dims; the host sums the two partial
outputs per batch and adds the (bias-folded) output bias.

Key scheduling/engine decisions:
- Projection inputs (enc, x, wq, wk, wv) ship as bf16 in chunk-major host
  layout ([128, chunks*cols]), so each tensor is one large contiguous DMA;
  only the cold-start K-proj inputs stream per chunk.
- Projections run d-major across all 8 PSUM banks; bank copy-backs alternate
  DVE/Act so the next phase's bank-0 chain never waits.
- Softmax denominators never touch the PE: exp'd score chunks (bf16)
  accumulate on DVE (2x mode), then one gpsimd partition_all_reduce per
  (q-half, head) produces the broadcast denominator, reciprocal on DVE.
- exp processes two key-chunks per Act instruction ([128,1024] spanning two
  PSUM banks) to keep Act throughput above the PE's score+AV rate.
- b_v is folded into the host-side output bias (b_eff = b_o + W_o @ b_v),
  removing all bias work from the attention loop.
- Attention is software-pipelined: one filler matmul per inner step keeps
  the PE busy while Act computes exp. Fillers for q-half 0 are the second
  half of the Q projection; fillers for q-half 1 are q-half 0's output
  projection. The PE is warmed with dummy matmuls on a memset tile so the
  p-state ramp completes before real work arrives.
"""

import math
import sys

import numpy as np

for _p in ("/opt/trn_rl_repo", "/root/.axon_site/_ro/trn_rl_repo"):
    if _p not in sys.path:
        sys.path.append(_p)

B = 4
LQ = 1024
LK = 2048
D = 1024
H = 8
DH = 128
P = 128
HPC = 4          # heads per core
OQ = HPC * DH    # 512 projected dims per core
NQ = 512         # matmul moving free dim
KC = D // P      # 8 contraction chunks for projections
LKC = LK // P    # 16 key chunks
HLK = LK // 2    # 1024, one lk-half of the encoder
N_CORES = 8

_BUILT = {}


def _build(masked):
    import concourse.bass as bass  # noqa: F401
    import concourse.tile as tile
    import concourse.mybir as mybir
    from concourse import bacc
    from concourse import bass_isa

    f32 = mybir.dt.float32
    f32r = mybir.dt.float32r
    bf16 = mybir.dt.bfloat16
    Exp = mybir.ActivationFunctionType.Exp

    nc = bacc.Bacc("TRN2", target_bir_lowering=False, debug=False,
                   num_devices=N_CORES)

    # chunk-major DRAM layouts: [...][p, d*cols + c] = chunk d, row p, col c
    xr = nc.dram_tensor("xr", [P, 2 * KC * NQ], bf16, kind="ExternalInput").ap()
    e0r = nc.dram_tensor("e0r", [P, KC * HLK], bf16, kind="ExternalInput").ap()
    e1r = nc.dram_tensor("e1r", [P, KC * HLK], bf16, kind="ExternalInput").ap()
    wqr = nc.dram_tensor("wqr", [P, KC * OQ], bf16, kind="ExternalInput").ap()
    wkr = nc.dram_tensor("wkr", [P, KC * OQ], bf16, kind="ExternalInput").ap()
    wvr = nc.dram_tensor("wvr", [P, KC * OQ], bf16, kind="ExternalInput").ap()
    wor = nc.dram_tensor("wor", [P, HPC * D], f32r, kind="ExternalInput").ap()
    bq_d = nc.dram_tensor("bq", [P, HPC], f32, kind="ExternalInput").ap()
    bk_d = nc.dram_tensor("bk", [P, HPC], f32, kind="ExternalInput").ap()
    if masked:
        maskT = nc.dram_tensor("maskT", [LK, LQ], f32, kind="ExternalInput").ap()
    out_d = nc.dram_tensor("out", [LQ, D], f32, kind="ExternalOutput").ap()

    with tile.TileContext(nc) as tc:
        with tc.tile_pool(name="persist", bufs=1) as persist:
            kT = [persist.tile([P, LK], f32r, name=f"kT{h}") for h in range(HPC)]
            qT = [persist.tile([P, LQ], f32r, name=f"qT{h}") for h in range(HPC)]
            vch = [persist.tile([P, OQ], bf16, name=f"v{j}") for j in range(LKC)]
            wq_all = persist.tile([P, KC * OQ], bf16, name="wq")
            wo_all = persist.tile([P, HPC * D], f32r, name="wo")
            x_all = persist.tile([P, 2 * KC * NQ], bf16, name="x")
            bq_sb = persist.tile([P, HPC], f32, name="bq")
            bk_sb = persist.tile([P, HPC], f32, name="bk")
            warm = persist.tile([P, 256], bf16, name="warm")

            wqc = [wq_all[:, d * OQ:(d + 1) * OQ] for d in range(KC)]
            woch = [wo_all[:, h * D:(h + 1) * D] for h in range(HPC)]
            xg0 = [x_all[:, d * NQ:(d + 1) * NQ] for d in range(KC)]
            xg1 = [x_all[:, (KC + d) * NQ:(KC + d + 1) * NQ] for d in range(KC)]

            with (
                tc.tile_pool(name="acc", bufs=1, space="PSUM") as accp,
                tc.tile_pool(name="wk", bufs=1) as wkp,
                tc.tile_pool(name="wv", bufs=1) as wvp,
                tc.tile_pool(name="e0p", bufs=1) as e0p,
                tc.tile_pool(name="e1p", bufs=1) as e1p,
            ):
                banks = [accp.tile([P, NQ], f32, name=f"bank{t}")
                         for t in range(8)]
                wk_all = wkp.tile([P, KC * OQ], bf16, name="wk")
                wv_all = wvp.tile([P, KC * OQ], bf16, name="wv")
                e0_all = e0p.tile([P, KC * HLK], bf16, name="e0")
                e1_all = e1p.tile([P, KC * HLK], bf16, name="e1")
                wkc = [wk_all[:, d * OQ:(d + 1) * OQ] for d in range(KC)]
                wvc = [wv_all[:, d * OQ:(d + 1) * OQ] for d in range(KC)]
                e0 = [e0_all[:, d * HLK:(d + 1) * HLK] for d in range(KC)]
                e1 = [e1_all[:, d * HLK:(d + 1) * HLK] for d in range(KC)]

                # --- PE warm-up: memset a tile (no DMA) and run dummy
                # matmuls so the p-state ramp happens before real work.
                nc.vector.memset(warm[:], 1.0)
                for _ in range(14):
                    nc.tensor.matmul(banks[0][0:16, 0:256], warm[:, 0:16],
                                     warm[:], start=True, stop=True)

                # --- DMA program: cold-start K inputs per chunk, the rest
                # as single contiguous transfers in need-order.
                for d in range(KC):
                    nc.sync.dma_start(wkc[d], wkr[:, d * OQ:(d + 1) * OQ])
                    nc.sync.dma_start(e0[d], e0r[:, d * HLK:(d + 1) * HLK])
                nc.sync.dma_start(bq_sb[:], bq_d[:])
                nc.sync.dma_start(bk_sb[:], bk_d[:])
                nc.sync.dma_start(wv_all[:], wvr[:])
                nc.sync.dma_start(e1_all[:], e1r[:])
                nc.sync.dma_start(wq_all[:], wqr[:])
                nc.sync.dma_start(x_all[:], xr[:])
                nc.sync.dma_start(wo_all[:], wor[:])

                def kproj_half(e, lh):
                    for d in range(KC):
                        for g in range(2):
                            for h in range(HPC):
                                nc.tensor.matmul(
                                    banks[g * 4 + h][:],
                                    wkc[d][:, h * DH:(h + 1) * DH],
                                    e[d][:, g * NQ:(g + 1) * NQ],
                                    start=(d == 0), stop=(d == KC - 1))
                    for i in range(8):
                        g, h = i // 4, i % 4
                        dst = kT[h][:, lh * HLK + g * NQ:lh * HLK + (g + 1) * NQ]
                        if i % 2 == 0:
                            nc.vector.tensor_scalar_add(
                                dst, banks[i][:], bk_sb[:, h:h + 1])
                        else:
                            nc.scalar.add(dst, banks[i][:], bk_sb[:, h:h + 1])

                def vproj_half(e, lh):
                    for d in range(KC):
                        for j8 in range(8):
                            nc.tensor.matmul(
                                banks[j8][:],
                                e[d][:, j8 * P:(j8 + 1) * P],
                                wvc[d],
                                start=(d == 0), stop=(d == KC - 1))
                    for j8 in range(8):
                        dst = vch[lh * 8 + j8][:]
                        if j8 % 2 == 0:
                            nc.vector.tensor_copy(dst, banks[j8][:])
                        else:
                            nc.scalar.copy(dst, banks[j8][:])

                kproj_half(e0, 0)
                vproj_half(e0, 0)
                kproj_half(e1, 1)
                vproj_half(e1, 1)

                # Q proj, query half 0, in two 2-head subphases so the
                # head-0/1 copy-backs finish before attention needs qT.
                for sub in range(2):
                    hs = (0, 1) if sub == 0 else (2, 3)
                    for d in range(KC):
                        for h in hs:
                            nc.tensor.matmul(
                                banks[h][:],
                                wqc[d][:, h * DH:(h + 1) * DH],
                                xg0[d],
                                start=(d == 0), stop=(d == KC - 1))
                    for h in hs:
                        nc.scalar.add(qT[h][:, 0:NQ], banks[h][:],
                                      bq_sb[:, h:h + 1])

            # ---- Attention, software-pipelined with filler matmuls.
            with (
                tc.tile_pool(name="pTp", bufs=6) as pTp,
                tc.tile_pool(name="dnp", bufs=2) as dnp,
                tc.tile_pool(name="maskp", bufs=16 if masked else 1) as maskp,
                tc.tile_pool(name="osb", bufs=4) as osb,
                tc.tile_pool(name="att", bufs=1) as attp,
                tc.tile_pool(name="pss", bufs=2, space="PSUM") as pss,
                tc.tile_pool(name="psa", bufs=2, space="PSUM") as psa,
            ):
                valsT = [attp.tile([P, LQ], f32r, name=f"valsT{h}")
                         for h in range(HPC)]

                def attn_q2(q2, fillers):
                    """h-loops for one query half. fillers: list of
                    callables, each emitting one PE matmul (+ its own
                    non-PE follow-ups); consumed one per inner step."""
                    fi = [0]

                    def fill():
                        if fi[0] < len(fillers):
                            fillers[fi[0]]()
                            fi[0] += 1

                    if masked:
                        mch = []
                        for j in range(LKC):
                            mt = maskp.tile([P, NQ], f32, name=f"m{j}")
                            nc.sync.dma_start(
                                mt[:], maskT[j * P:(j + 1) * P,
                                             q2 * NQ:(q2 + 1) * NQ])
                            mch.append(mt)

                    for h in range(HPC):
                        qs = qT[h][:, q2 * NQ:(q2 + 1) * NQ]
                        ps_v = psa.tile([P, NQ], f32, name="ps_v")
                        pT = [None] * 8
                        acc = None

                        def spair(g):
                            t = pss.tile([P, 2 * NQ], f32, name="ps_s")
                            for jj in range(2):
                                j = 2 * g + jj
                                nc.tensor.matmul(
                                    t[:, jj * NQ:(jj + 1) * NQ],
                                    kT[h][:, j * P:(j + 1) * P],
                                    qs, start=True, stop=True)
                            return t

                        def do_exp(g, t):
                            if masked:
                                for jj in range(2):
                                    j = 2 * g + jj
                                    nc.vector.tensor_add(
                                        t[:, jj * NQ:(jj + 1) * NQ],
                                        t[:, jj * NQ:(jj + 1) * NQ],
                                        mch[j][:])
                            p = pTp.tile([P, 2 * NQ], bf16, name="pT")
                            nc.scalar.activation(p[:], t[:], Exp)
                            pT[g] = p

                        def avpair(g):
                            for jj in range(2):
                                j = 2 * g + jj
                                nc.tensor.matmul(
                                    ps_v[:],
                                    vch[j][:, h * DH:(h + 1) * DH],
                                    pT[g][:, jj * NQ:(jj + 1) * NQ],
                                    start=(j == 0), stop=(j == LKC - 1))

                        st = [spair(0)]
                        st.append(spair(1))
                        for g in range(8):
                            do_exp(g, st[g])
                            if g + 2 < 8:
                                st.append(spair(g + 2))
                            fill()
                            avpair(g)
                            # denominator accumulation on DVE (bf16 2x)
                            p = pT[g]
                            if g == 0:
                                acc = dnp.tile([P, NQ], bf16, name="dacc")
                                nc.vector.tensor_add(
                                    acc[:], p[:, 0:NQ], p[:, NQ:2 * NQ])
                            else:
                                nc.vector.tensor_add(
                                    acc[:], acc[:], p[:, 0:NQ])
                                nc.vector.tensor_add(
                                    acc[:], acc[:], p[:, NQ:2 * NQ])
                        dbc = dnp.tile([P, NQ], f32, name="dbc")
                        nc.gpsimd.partition_all_reduce(
                            dbc[:], acc[:], channels=P,
                            reduce_op=bass_isa.ReduceOp.add)
                        rr = dnp.tile([P, NQ], f32, name="rr")
                        nc.vector.reciprocal(rr[:], dbc[:])
                        nc.vector.tensor_mul(
                            valsT[h][:, q2 * NQ:(q2 + 1) * NQ],
                            ps_v[:], rr[:])

                def outproj_fillers(q2, out_pool):
                    """32 closures: 8 chains x 4 head-parts; the last part
                    of each chain emits the copy-back (alternating DVE/Act)
                    and the store DMA. The final chain splits its copy+DMA
                    in half to shorten the kernel tail."""
                    fillers = []
                    state = {}
                    for c in range(8):
                        lqc = q2 * 4 + c // 2
                        o2 = c % 2
                        last = (q2 == 1 and c == 7)
                        for hh in range(HPC):
                            def f(c=c, lqc=lqc, o2=o2, hh=hh, last=last):
                                if hh == 0:
                                    state[c] = out_pool.tile(
                                        [P, NQ], f32, name="pso")
                                po = state[c]
                                nc.tensor.matmul(
                                    po[:],
                                    valsT[hh][:, lqc * P:(lqc + 1) * P],
                                    woch[hh][:, o2 * NQ:(o2 + 1) * NQ],
                                    start=(hh == 0), stop=(hh == HPC - 1))
                                if hh == HPC - 1:
                                    dst = out_d[lqc * P:(lqc + 1) * P,
                                                o2 * NQ:(o2 + 1) * NQ]
                                    if last:
                                        for half in range(2):
                                            sl = slice(half * (NQ // 2),
                                                       (half + 1) * (NQ // 2))
                                            ot = osb.tile([P, NQ // 2], f32,
                                                          name="ot")
                                            nc.vector.tensor_copy(
                                                ot[:], po[:, sl])
                                            nc.sync.dma_start(dst[:, sl],
                                                              ot[:])
                                    else:
                                        ot = osb.tile([P, NQ], f32, name="ot")
                                        if c % 2 == 0:
                                            nc.vector.tensor_copy(
                                                ot[:], po[:])
                                        else:
                                            nc.scalar.copy(ot[:], po[:])
                                        nc.sync.dma_start(dst, ot[:])
                            fillers.append(f)
                    return fillers

                def qg1_fillers(psq):
                    """32 closures: 4 head d-chains; last d emits the
                    qT copy-back (DVE, since Act is saturated)."""
                    fillers = []
                    state = {}
                    for hh in range(HPC):
                        for d in range(KC):
                            def f(hh=hh, d=d):
                                if d == 0:
                                    state[hh] = psq.tile(
                                        [P, NQ], f32, name="psq")
                                pq = state[hh]
                                nc.tensor.matmul(
                                    pq[:],
                                    wqc[d][:, hh * DH:(hh + 1) * DH],
                                    xg1[d],
                                    start=(d == 0), stop=(d == KC - 1))
                                if d == KC - 1:
                                    nc.vector.tensor_scalar_add(
                                        qT[hh][:, NQ:2 * NQ], pq[:],
                                        bq_sb[:, hh:hh + 1])
                            fillers.append(f)
                    return fillers

                with tc.tile_pool(name="psq", bufs=2, space="PSUM") as psq:
                    attn_q2(0, qg1_fillers(psq))
                with tc.tile_pool(name="pso", bufs=2, space="PSUM") as pso:
                    attn_q2(1, outproj_fillers(0, pso))
                    # output projection for query half 1, inline.
                    for f in outproj_fillers(1, pso):
                        f()

    nc.compile()
    return nc


def _get_built(masked):
    if masked not in _BUILT:
        _BUILT[masked] = _build(masked)
    return _BUILT[masked]


def _chunk_major(a, n_chunks):
    """[n_chunks*P, C] -> [P, n_chunks*C] with [p, d*C+c] = a[d*P+p, c]."""
    C = a.shape[1]
    return np.ascontiguousarray(
        a.reshape(n_chunks, P, C).transpose(1, 0, 2).reshape(P, n_chunks * C))


def _shard_inputs(inputs, masked):
    import ml_dtypes

    bf16 = ml_dtypes.bfloat16

    x = np.asarray(inputs["mhca_input"], np.float32)
    enc = np.asarray(inputs["encoder_output"], np.float32)
    mask = np.asarray(inputs["cross_mask"], np.float32)
    W_kv = np.asarray(inputs["W_kv"], np.float32)
    b_kv = np.asarray(inputs["b_kv"], np.float32)
    W_q = np.asarray(inputs["W_q"], np.float32)
    b_q = np.asarray(inputs["b_q"], np.float32)
    W_o = np.asarray(inputs["W_o"], np.float32)

    scale = 1.0 / math.sqrt(DH)
    in_maps = []
    for c in range(N_CORES):
        b = c // 2
        g = c % 2
        heads = list(range(g * HPC, (g + 1) * HPC))
        sl = slice(g * OQ, (g + 1) * OQ)
        k_rows = np.concatenate(
            [W_kv[h * 2 * DH:h * 2 * DH + DH] for h in heads], 0)
        v_rows = np.concatenate(
            [W_kv[h * 2 * DH + DH:(h + 1) * 2 * DH] for h in heads], 0)
        xT = x[b].T                                   # [D, LQ]
        encT = enc[b].T                               # [D, LK]
        m = {
            "xr": _chunk_major(
                np.concatenate([xT[:, :NQ], xT[:, NQ:]], 0), 2 * KC
            ).astype(bf16),
            "e0r": _chunk_major(encT[:, :HLK], KC).astype(bf16),
            "e1r": _chunk_major(encT[:, HLK:], KC).astype(bf16),
            "wqr": _chunk_major((W_q[sl] * scale).T, KC).astype(bf16),
            "wkr": _chunk_major(k_rows.T, KC).astype(bf16),
            "wvr": _chunk_major(v_rows.T, KC).astype(bf16),
            "wor": _chunk_major(np.ascontiguousarray(W_o[:, sl].T), HPC),
            "bq": np.ascontiguousarray((b_q[sl] * scale).reshape(HPC, DH).T),
            "bk": np.ascontiguousarray(
                np.stack([b_kv[h * 2 * DH:h * 2 * DH + DH] for h in heads], 1)),
        }
        if masked:
            m["maskT"] = np.ascontiguousarray(mask[b].T)
        in_maps.append(m)
    return in_maps


def kernel(mhca_input, encoder_output, cross_mask, W_kv, b_kv, W_q, b_q, W_o,
           b_o):
    from concourse.bass_utils import run_bass_kernel_spmd

    inputs = {
        "mhca_input": mhca_input, "encoder_output": encoder_output,
        "cross_mask": cross_mask, "W_kv": W_kv, "b_kv": b_kv, "W_q": W_q,
        "b_q": b_q, "W_o": W_o,
    }
    b_kv = np.asarray(b_kv, np.float32)
    b_o = np.asarray(b_o, np.float32)
    W_o_np = np.asarray(W_o, np.float32)
    # v-bias folds into the output bias: out += W_o @ b_v + b_o
    b_v_vec = np.concatenate(
        [b_kv[h * 2 * DH + DH:(h + 1) * 2 * DH] for h in range(H)], 0)
    b_eff = b_o + W_o_np @ b_v_vec
    masked = bool(np.any(np.asarray(cross_mask)))
    nc = _get_built(masked)
    in_maps = _shard_inputs(inputs, masked)

    res = run_bass_kernel_spmd(nc, in_maps, core_ids=list(range(N_CORES)))
    outs = [res.results[c]["out"] for c in range(N_CORES)]
    full = np.stack([outs[2 * b] + outs[2 * b + 1] for b in range(B)], 0)
    return (full + b_eff[None, None, :]).astype(np.float32)
